# revision 12
# baseline (speedup 1.0000x reference)
"""HGCN encoder forward on 8 Trainium2 NeuronCores.

Computation (per batch b):
    w_abs = |gelu(states @ W1.T + b1) @ W2.T + b2|          (E,)  [host, tiny]
    d[n]    = sum_e H[n,e] * w_abs[e]                        (N,)
    dinv[n] = rsqrt(d[n])  (d > 0 always for these inputs)
    X[e,dd] = leaky_relu( sum_n (H[n,e]*w_abs[e]) * (dinv[n]*nf[n,dd]) )

Sharding: core c -> (batch b = c//2, node-half c%2) so each core owns
4096 full node rows (H slice 32 MiB).  The kernel is DMA-bound: H must
stream through at ~360 GB/s (~90 us/core), so every other engine is
kept off the critical path:

  * Per 128-node tile one fused DVE scalar_tensor_tensor produces both
    hw = H*w_abs (matmul rhs, rounded to float32r) and the row-sum d.
  * float32r matmuls run at 1 cycle/row (fp32 is 4) -- PE never backs
    up the hw pool, so DMA streams gap-free.
  * The rsqrt chain (ACT sqrt -> DVE reciprocal -> ACT mul) for tile i
    is software-pipelined one tile behind.  sqrt(i) is emitted AFTER
    finish(i-1): the tile framework's counting semaphores make a DVE
    instruction wait on every earlier-emitted ACT instruction, so
    emitting sqrt(i) first would stall recip(i-1) ~400ns per tile.
  * w_abs arrives as one bf16 row (4 KB) and is broadcast to 128
    partitions by the idle PE (ones x w outer-product into the 4 spare
    PSUM banks) -- no 1 MiB broadcast DMA on the H stream, and no
    GPSIMD ucode mode-switch (whose drain costs ~12 us at startup).
  * nf arrives in a single DMA; output drains per-bank, copies
    alternating between ACT and DVE, as each accumulation finishes.

Host sums the two per-batch partials and applies leaky_relu.
"""

import sys

for _p in ("/opt/trn_rl_repo",):
    if _p not in sys.path:
        sys.path.insert(0, _p)

import numpy as np

B, N, E, S, D = 4, 8192, 2048, 64, 16
NCORES = 8
NSHARD = N // 2          # nodes per core
NT = NSHARD // 128       # 32 tiles per core
ECH = 512                # e-chunk per matmul (one PSUM bank, fp32 max)
NJ = E // ECH            # 4 matmuls per tile

_CACHE = {}


def _build_nc():
    import concourse.bass as bass  # noqa: F401
    import concourse.mybir as mybir
    import concourse.tile as tile
    from concourse import bacc

    f32 = mybir.dt.float32
    f32r = mybir.dt.float32r
    bf16 = mybir.dt.bfloat16
    nc = bacc.Bacc(
        "TRN2",
        target_bir_lowering=False,
        debug=False,
        num_devices=NCORES,
    )
    hg = nc.dram_tensor("hg", [NT, 128, E], f32, kind="ExternalInput").ap()
    nf = nc.dram_tensor("nf", [128, NT * D], f32, kind="ExternalInput").ap()
    wr = nc.dram_tensor("wr", [1, E], bf16, kind="ExternalInput").ap()
    y = nc.dram_tensor("y", [D, E], f32, kind="ExternalOutput").ap()

    with tile.TileContext(nc) as tc:
        with (
            tc.tile_pool(name="hpool", bufs=8) as hpool,
            tc.tile_pool(name="hwpool", bufs=6) as hwpool,
            tc.tile_pool(name="wpool", bufs=1) as wpool,
            tc.tile_pool(name="small", bufs=6) as small,
            tc.tile_pool(name="psum", bufs=1, space="PSUM") as psum_pool,
        ):
            w_full = wpool.tile([128, E], f32, tag="wfull")
            nf_all = wpool.tile([128, NT * D], f32, tag="nfall")
            y_tile = wpool.tile([D, E], f32, tag="y")

            # [D, 512] accumulators, one PSUM bank per e-chunk. lhsT = s
            # (cheap 16-col weight load), hw streams as the moving operand.
            # Interleaved accumulation groups are safe across DIFFERENT
            # banks (same-bank interleaving corrupts results on HW).
            accs = [
                psum_pool.tile([D, ECH], f32, tag=f"acc{j}", name=f"acc{j}")
                for j in range(NJ)
            ]

            AF = mybir.ActivationFunctionType
            for i in range(NT):
                if i == 0:
                    # 4 KB w row first: it gates the whole compute chain
                    # and delays the H stream by only ~30ns.
                    w_row = wpool.tile([1, E], bf16, tag="wrow")
                    nc.sync.dma_start(w_row[:], wr[:])
                h_tile = hpool.tile([128, E], f32, tag="h")
                nc.sync.dma_start(h_tile[:], hg[i])
                if i == 0:
                    nc.sync.dma_start(nf_all[:], nf[:])
                    # w broadcast, built while H tile 0 is in flight.
                    # PE outer-product: ones[1,128].T @ wr[1,E] per 512-wide
                    # chunk into the 4 spare PSUM banks, then copied to SBUF
                    # on ACT/DVE.  bf16 runs at 1 cycle/row even from a cold
                    # PE p-state; fp32 would take ~10us here.
                    ones_t = wpool.tile([1, 128], bf16, tag="ones")
                    nc.vector.memset(ones_t[:], 1.0)
                    wps = [
                        psum_pool.tile(
                            [128, ECH], f32, tag=f"wb{j}", name=f"wb{j}"
                        )
                        for j in range(NJ)
                    ]
                    for j in range(NJ):
                        ch = slice(j * ECH, (j + 1) * ECH)
                        nc.tensor.matmul(
                            wps[j][:], lhsT=ones_t[:], rhs=w_row[:, ch]
                        )
                        if j % 2 == 0:
                            nc.scalar.copy(w_full[:, ch], wps[j][:])
                        else:
                            nc.vector.tensor_copy(w_full[:, ch], wps[j][:])

                # float32r output: same 4 bytes, rounded so the PE runs the
                # matmul at 1 cycle/row (plain fp32 is 4 cycles/row).
                hw_tile = hwpool.tile([128, E], f32r, tag="hw")
                d_t = small.tile([128, 1], f32, tag="d")
                # hw = (H * 1.0) * w_abs ; d = sum_e hw   (single DVE pass)
                nc.vector.scalar_tensor_tensor(
                    out=hw_tile[:],
                    in0=h_tile[:],
                    scalar=1.0,
                    in1=w_full[:],
                    op0=mybir.AluOpType.mult,
                    op1=mybir.AluOpType.mult,
                    accum_out=d_t[:],
                )
                # dinv = d^-1/2 = Exp(-0.5*Ln(d)), entirely on ACT: DVE runs
                # nothing but back-to-back STTs.  A DVE reciprocal would make
                # the tile scheduler insert a DVE->ACT->DVE round-trip wait
                # (~440ns) between consecutive STTs, pushing DVE past the
                # DMA pace.  (ACT Rsqrt is API-banned for accuracy.)
                ln_t = small.tile([128, 1], f32, tag="ln")
                nc.scalar.activation(ln_t[:], d_t[:], AF.Ln)
                dinv_t = small.tile([128, 1], f32, tag="dinv")
                nc.scalar.activation(dinv_t[:], ln_t[:], AF.Exp, scale=-0.5)
                s_tile = small.tile([128, D], f32r, tag="s")
                nc.scalar.mul(
                    s_tile[:], nf_all[:, i * D : (i + 1) * D], dinv_t[:]
                )
                for j in range(NJ):
                    ch = slice(j * ECH, (j + 1) * ECH)
                    nc.tensor.matmul(
                        accs[j][:],
                        lhsT=s_tile[:],
                        rhs=hw_tile[:, ch],
                        start=(i == 0),
                        stop=(i == NT - 1),
                    )
                    if i == NT - 1:
                        # drain each bank as soon as its accumulation ends;
                        # alternate copy engines so the tail pipelines
                        if j % 2 == 0:
                            nc.scalar.copy(y_tile[:, ch], accs[j][:])
                        else:
                            nc.vector.tensor_copy(y_tile[:, ch], accs[j][:])
                        nc.sync.dma_start(y[:, ch], y_tile[:, ch])

    nc.compile()
    return nc


def _get_nc():
    if "nc" not in _CACHE:
        _CACHE["nc"] = _build_nc()
    return _CACHE["nc"]


def _host_wabs(states, W1, b1, W2, b2):
    from scipy.special import erf

    st = states.astype(np.float64)
    h = st @ W1.astype(np.float64).T + b1.astype(np.float64)
    h = h * 0.5 * (1.0 + erf(h / np.sqrt(2.0)))
    w = h @ W2.astype(np.float64).T + b2.astype(np.float64)
    return np.abs(w).astype(np.float32)  # (B, E)


def _make_in_maps(node_features, hyper_graph, w_abs):
    import ml_dtypes

    in_maps = []
    for c in range(NCORES):
        b, half = c // 2, c % 2
        sl = slice(half * NSHARD, (half + 1) * NSHARD)
        hg_c = np.ascontiguousarray(hyper_graph[b, sl]).reshape(NT, 128, E)
        nf_c = np.ascontiguousarray(
            node_features[b, sl]
            .reshape(NT, 128, D)
            .transpose(1, 0, 2)
            .reshape(128, NT * D)
        )
        wr_c = np.ascontiguousarray(
            w_abs[b][None, :].astype(ml_dtypes.bfloat16)
        )
        in_maps.append({"hg": hg_c, "nf": nf_c, "wr": wr_c})
    return in_maps


def kernel(**inputs):
    from concourse.bass_utils import run_bass_kernel_spmd

    node_features = np.asarray(inputs["node_features"], dtype=np.float32)
    hyper_graph = np.asarray(inputs["hyper_graph"], dtype=np.float32)
    states = np.asarray(inputs["states"], dtype=np.float32)
    W1 = np.asarray(inputs["W1"], dtype=np.float32)
    b1 = np.asarray(inputs["b1"], dtype=np.float32)
    W2 = np.asarray(inputs["W2"], dtype=np.float32)
    b2 = np.asarray(inputs["b2"], dtype=np.float32)

    w_abs = _host_wabs(states, W1, b1, W2, b2)
    in_maps = _make_in_maps(node_features, hyper_graph, w_abs)

    nc = _get_nc()
    res = run_bass_kernel_spmd(nc, in_maps, core_ids=list(range(NCORES)))

    X = np.empty((B, E, D), dtype=np.float32)
    for b in range(B):
        p = res.results[2 * b]["y"] + res.results[2 * b + 1]["y"]  # (D, E)
        xb = p.T
        X[b] = np.where(xb >= 0, xb, np.float32(0.1) * xb)
    return X


# revision 15
# speedup vs baseline: 1.1731x; 1.1731x over previous
"""HGCN encoder forward on 8 Trainium2 NeuronCores.

Computation (per batch b):
    w_abs = |gelu(states @ W1.T + b1) @ W2.T + b2|          (E,)  [host, tiny]
    d[n]    = sum_e H[n,e] * w_abs[e]                        (N,)
    dinv[n] = rsqrt(d[n])  (d > 0 always for these inputs)
    X[e,dd] = leaky_relu( sum_n (H[n,e]*w_abs[e]) * (dinv[n]*nf[n,dd]) )

Sharding: core c -> (batch b = c//2, node-half c%2) so each core owns
4096 full node rows (H slice 32 MiB).  The kernel is DMA-bound: H must
stream through at ~360-390 GB/s (~88 us/core), so every other engine
is kept off the critical path:

  * Per 128-node tile one fused DVE scalar_tensor_tensor produces both
    hw = H*w_abs (matmul rhs, rounded to float32r) and the row-sum d.
  * float32r matmuls run at 1 cycle/row (fp32 is 4) -- PE never backs
    up the hw pool.
  * The rsqrt chain (ACT sqrt -> DVE reciprocal) is batched over 4
    tiles ([128,4] ops): the tile scheduler statically orders the DVE
    reciprocal right after its sqrt (its DMA model is pessimistic, so
    it never hoists the next STT first), which costs a ~440ns
    ACT-round-trip stall on DVE -- batching pays it once per 4 tiles
    instead of every tile, keeping DVE (~2.55us/tile) under the DMA
    pace (~2.7us/tile).  The last 4 tiles run per-tile so the final
    PSUM drain doesn't sit behind a 16-matmul burst.
  * w_abs arrives as TWO bf16 rows (hi + lo residual, 8 KB total) and
    is broadcast exactly by the idle PE: ones[1,128].T @ hi accumulated
    with ones.T @ lo in PSUM reconstructs fp32 w to ~1e-5 -- no 1 MiB
    broadcast DMA on the H stream, no GPSIMD ucode mode-switch (~12us
    drain), and bf16 matmuls run 1 cycle/row even from a cold PE.
  * nf arrives in a single DMA; the output drains per-bank (copies
    alternating ACT/DVE) as each accumulation finishes.

Host sums the two per-batch partials and applies leaky_relu.
"""

import sys

for _p in ("/opt/trn_rl_repo",):
    if _p not in sys.path:
        sys.path.insert(0, _p)

import numpy as np

B, N, E, S, D = 4, 8192, 2048, 64, 16
NCORES = 8
NSHARD = N // 2          # nodes per core
NT = NSHARD // 128       # 32 tiles per core
ECH = 512                # e-chunk per matmul (one PSUM bank, fp32 max)
NJ = E // ECH            # 4 matmuls per tile
DBLK = 4                 # tiles per batched-rsqrt block
NBLK = NT - DBLK         # tiles 0..27 run in blocks, 28..31 per-tile

_CACHE = {}


def _build_nc():
    import concourse.bass as bass  # noqa: F401
    import concourse.mybir as mybir
    import concourse.tile as tile
    from concourse import bacc

    f32 = mybir.dt.float32
    f32r = mybir.dt.float32r
    bf16 = mybir.dt.bfloat16
    nc = bacc.Bacc(
        "TRN2",
        target_bir_lowering=False,
        debug=False,
        num_devices=NCORES,
    )
    hg = nc.dram_tensor("hg", [NT, 128, E], f32, kind="ExternalInput").ap()
    nf = nc.dram_tensor("nf", [128, NT * D], f32, kind="ExternalInput").ap()
    wr = nc.dram_tensor("wr", [2, E], bf16, kind="ExternalInput").ap()
    y = nc.dram_tensor("y", [D, E], f32, kind="ExternalOutput").ap()

    with tile.TileContext(nc) as tc:
        with (
            tc.tile_pool(name="hpool", bufs=8) as hpool,
            tc.tile_pool(name="hwpool", bufs=8) as hwpool,
            tc.tile_pool(name="wpool", bufs=1) as wpool,
            tc.tile_pool(name="small", bufs=6) as small,
            tc.tile_pool(name="psum", bufs=1, space="PSUM") as psum_pool,
        ):
            w_full = wpool.tile([128, E], f32, tag="wfull")
            nf_all = wpool.tile([128, NT * D], f32, tag="nfall")
            y_tile = wpool.tile([D, E], f32, tag="y")

            # [D, 512] accumulators, one PSUM bank per e-chunk. lhsT = s
            # (cheap 16-col weight load), hw streams as the moving operand.
            # Interleaved accumulation groups are safe across DIFFERENT
            # banks (same-bank interleaving corrupts results on HW).
            accs = [
                psum_pool.tile([D, ECH], f32, tag=f"acc{j}", name=f"acc{j}")
                for j in range(NJ)
            ]

            def emit_tile_tail(hw_tile, dinv_ap, i):
                # s = dinv * nf, then the tile's 4 PSUM-accumulating matmuls
                s_tile = small.tile([128, D], f32r, tag="s")
                nc.scalar.mul(
                    s_tile[:], nf_all[:, i * D : (i + 1) * D], dinv_ap
                )
                for j in range(NJ):
                    ch = slice(j * ECH, (j + 1) * ECH)
                    nc.tensor.matmul(
                        accs[j][:],
                        lhsT=s_tile[:],
                        rhs=hw_tile[:, ch],
                        start=(i == 0),
                        stop=(i == NT - 1),
                    )
                    if i == NT - 1:
                        # drain each bank as soon as its accumulation ends;
                        # alternate copy engines so the tail pipelines
                        if j % 2 == 0:
                            nc.scalar.copy(y_tile[:, ch], accs[j][:])
                        else:
                            nc.vector.tensor_copy(y_tile[:, ch], accs[j][:])
                        nc.sync.dma_start(y[:, ch], y_tile[:, ch])

            dblk_t = None
            pend = []
            for i in range(NT):
                if i == 0:
                    # 8 KB w rows first: they gate the whole compute chain
                    # and delay the H stream by only ~60ns.  (Two separate
                    # tiles: a matmul rhs must start at partition 0.)
                    w_hi = wpool.tile([1, E], bf16, tag="whi")
                    nc.sync.dma_start(w_hi[:], wr[0:1, :])
                    w_lo = wpool.tile([1, E], bf16, tag="wlo")
                    nc.sync.dma_start(w_lo[:], wr[1:2, :])
                h_tile = hpool.tile([128, E], f32, tag="h")
                nc.sync.dma_start(h_tile[:], hg[i])
                if i == 0:
                    nc.sync.dma_start(nf_all[:], nf[:])
                    ones_t = wpool.tile([1, 128], bf16, tag="ones")
                    nc.vector.memset(ones_t[:], 1.0)
                    wps = [
                        psum_pool.tile(
                            [128, ECH], f32, tag=f"wb{j}", name=f"wb{j}"
                        )
                        for j in range(NJ)
                    ]
                    for j in range(NJ):
                        ch = slice(j * ECH, (j + 1) * ECH)
                        nc.tensor.matmul(
                            wps[j][:],
                            lhsT=ones_t[:],
                            rhs=w_hi[:, ch],
                            start=True,
                            stop=False,
                        )
                        nc.tensor.matmul(
                            wps[j][:],
                            lhsT=ones_t[:],
                            rhs=w_lo[:, ch],
                            start=False,
                            stop=True,
                        )
                        if j % 2 == 0:
                            nc.scalar.copy(w_full[:, ch], wps[j][:])
                        else:
                            nc.vector.tensor_copy(w_full[:, ch], wps[j][:])

                blocked = i < NBLK
                if blocked and i % DBLK == 0:
                    dblk_t = small.tile([128, DBLK], f32, tag="d4")
                    pend = []

                # float32r output: same 4 bytes, rounded so the PE runs the
                # matmul at 1 cycle/row (plain fp32 is 4 cycles/row).
                hw_tile = hwpool.tile([128, E], f32r, tag="hw")
                if blocked:
                    d_ap = dblk_t[:, i % DBLK : i % DBLK + 1]
                else:
                    d_t = small.tile([128, 1], f32, tag="d")
                    d_ap = d_t[:]
                # hw = (H * 1.0) * w_abs ; d = sum_e hw   (single DVE pass)
                nc.vector.scalar_tensor_tensor(
                    out=hw_tile[:],
                    in0=h_tile[:],
                    scalar=1.0,
                    in1=w_full[:],
                    op0=mybir.AluOpType.mult,
                    op1=mybir.AluOpType.mult,
                    accum_out=d_ap,
                )

                if blocked:
                    pend.append((hw_tile, i))
                    if i % DBLK == DBLK - 1:
                        sq4 = small.tile([128, DBLK], f32, tag="sq4")
                        nc.scalar.sqrt(sq4[:], dblk_t[:])
                        dinv4 = small.tile([128, DBLK], f32, tag="dinv4")
                        nc.vector.reciprocal(dinv4[:], sq4[:])
                        for hw_t, ti in pend:
                            k = ti % DBLK
                            emit_tile_tail(hw_t, dinv4[:, k : k + 1], ti)
                        pend = []
                else:
                    sq_t = small.tile([128, 1], f32, tag="sq")
                    nc.scalar.sqrt(sq_t[:], d_ap)
                    dinv_t = small.tile([128, 1], f32, tag="dinv")
                    nc.vector.reciprocal(dinv_t[:], sq_t[:])
                    emit_tile_tail(hw_tile, dinv_t[:], i)

    nc.compile()
    return nc


def _get_nc():
    if "nc" not in _CACHE:
        _CACHE["nc"] = _build_nc()
    return _CACHE["nc"]


def _host_wabs(states, W1, b1, W2, b2):
    from scipy.special import erf

    st = states.astype(np.float64)
    h = st @ W1.astype(np.float64).T + b1.astype(np.float64)
    h = h * 0.5 * (1.0 + erf(h / np.sqrt(2.0)))
    w = h @ W2.astype(np.float64).T + b2.astype(np.float64)
    return np.abs(w).astype(np.float32)  # (B, E)


def _make_in_maps(node_features, hyper_graph, w_abs):
    import ml_dtypes

    in_maps = []
    for c in range(NCORES):
        b, half = c // 2, c % 2
        sl = slice(half * NSHARD, (half + 1) * NSHARD)
        hg_c = np.ascontiguousarray(hyper_graph[b, sl]).reshape(NT, 128, E)
        nf_c = np.ascontiguousarray(
            node_features[b, sl]
            .reshape(NT, 128, D)
            .transpose(1, 0, 2)
            .reshape(128, NT * D)
        )
        # exact-ish w as a bf16 (hi, lo) pair: hi + lo == w to ~1e-5 rel.
        hi = w_abs[b].astype(ml_dtypes.bfloat16)
        lo = (w_abs[b] - hi.astype(np.float32)).astype(ml_dtypes.bfloat16)
        wr_c = np.ascontiguousarray(np.stack([hi, lo], axis=0))
        in_maps.append({"hg": hg_c, "nf": nf_c, "wr": wr_c})
    return in_maps


def kernel(**inputs):
    from concourse.bass_utils import run_bass_kernel_spmd

    node_features = np.asarray(inputs["node_features"], dtype=np.float32)
    hyper_graph = np.asarray(inputs["hyper_graph"], dtype=np.float32)
    states = np.asarray(inputs["states"], dtype=np.float32)
    W1 = np.asarray(inputs["W1"], dtype=np.float32)
    b1 = np.asarray(inputs["b1"], dtype=np.float32)
    W2 = np.asarray(inputs["W2"], dtype=np.float32)
    b2 = np.asarray(inputs["b2"], dtype=np.float32)

    w_abs = _host_wabs(states, W1, b1, W2, b2)
    in_maps = _make_in_maps(node_features, hyper_graph, w_abs)

    nc = _get_nc()
    res = run_bass_kernel_spmd(nc, in_maps, core_ids=list(range(NCORES)))

    X = np.empty((B, E, D), dtype=np.float32)
    for b in range(B):
        p = res.results[2 * b]["y"] + res.results[2 * b + 1]["y"]  # (D, E)
        xb = p.T
        X[b] = np.where(xb >= 0, xb, np.float32(0.1) * xb)
    return X
